# revision 37
# baseline (speedup 1.0000x reference)
"""Trainium2 Bass kernel for nn_Attention (B=2, S=2048, D=1024, H=16, hd=64).

Sharding: 8 cores = 2 batches x 4 head-groups (4 heads / 256 dims each).
Host sums the 4 partial output projections per batch and adds wo_b.

v6 design:
  - score MMs contract an honest K=64 per head via tile_position ROW
    groups (parity-0 heads rows 0-63, parity-1 rows 64-127); no
    zero-padded q copies. Head->ss slot permutation [0,2,1,3] keeps the
    row-group pair's outputs in different PSUM banks.
    (COL-tiled matmuls -- M<128 via tile_position[1] -- produce garbage
    on this toolchain; row tiling is fine. Verified empirically.)
  - RoPE full-width: P2 = 32-row-block-swapped copy of P1 (via sbuf-sbuf
    DMA), then q' = P1*cos + P2*(+-sin) -- 3 wide DVE ops per slice.
  - tq groups of 256; scores for all 4 heads of a (tqg, tkt) slot land in
    one [128, 4, 256] PSUM tile -> ONE exp per slot on ACT. All exps
    resolve to a single ACT table set (see _build's table patch).
  - PV token-major: out[tq 128, 65] via lhsT=probs chunk, rhs=v (ones
    column = softmax denominator -> per-PARTITION denominators, so the
    DVE reciprocal has free-size 2; DVE recip runs at 1/8 rate so wide
    reciprocals are catastrophic).
  - attn (token-major) -> dims-major via DMA-transpose XBAR, spread
    across the two HWDGE queues (sync + scalar); output projection per
    tqg interleaved with later attention; bf16 partials.
  - causal: skip above-diagonal tiles, 128-col trim on odd diag slots,
    triangle mask multiply on DVE.
  - prologue: xT loaded by 512-col phases on sync while weights/rope
    tables ride gpsimd, so proj/rope/attention(0) start ASAP.
"""

import sys

sys.path.insert(0, "/opt/trn_rl_repo")

import numpy as np
import ml_dtypes

B, S, D = 2, 2048, 1024
H = 16
HD = 64
HPC = 4          # heads per core
DPC = HPC * HD   # 256 dims per core
NCORES = 8
NKT = D // 128   # 8 k-tiles over d_in
NPH = 4          # projection phases (512 cols each)
NTQG = 8         # tq groups of 256
NTOK = S // 128  # 16 token tiles
SLOT_OF = [0, 2, 1, 3]  # head -> ss/pt slot (concurrent pairs in diff banks)

_BUILT = {}


def _build():
    import concourse.bass as bass
    import concourse.mybir as mybir
    import concourse.tile as tile
    from concourse import bacc

    dt = mybir.dt
    AF = mybir.ActivationFunctionType
    ALU = mybir.AluOpType

    nc = bacc.Bacc()

    # Exp and Ln both live in the natural_log_exp_and_others ACT table set.
    # The table chooser maps each func to the first set containing it, which
    # would alternate exp_and_others <-> natural_log (a ~2.7us table reload
    # per switch). Drop those funcs from the earlier sets (cached dict is
    # shared) so every activation resolves to the combined set: one load.
    from concourse.hw_specs import get_activation_tables
    tabs = get_activation_tables(nc.m.arch)
    if "natural_log_exp_and_others" in tabs:
        tabs["exp_and_others"].discard(AF.Exp)
        tabs["natural_log"].discard(AF.Ln)

    f32, bf16 = dt.float32, dt.bfloat16

    xT = nc.declare_dram_parameter("xT", [D, S], bf16, isOutput=False)
    wq = nc.declare_dram_parameter("wq", [D, DPC], bf16, isOutput=False)
    wk = nc.declare_dram_parameter("wk", [D, DPC], bf16, isOutput=False)
    wv = nc.declare_dram_parameter("wv", [D, DPC], bf16, isOutput=False)
    bq = nc.declare_dram_parameter("bq", [2, 128], f32, isOutput=False)
    bk = nc.declare_dram_parameter("bk", [2, 128], f32, isOutput=False)
    bv = nc.declare_dram_parameter("bv", [1, DPC], f32, isOutput=False)
    cs = nc.declare_dram_parameter("cs", [128, S], bf16, isOutput=False)
    sn = nc.declare_dram_parameter("sn", [128, S], bf16, isOutput=False)
    wo = nc.declare_dram_parameter("wo", [DPC, D], bf16, isOutput=False)
    msk = nc.declare_dram_parameter("msk", [128, HPC * 128], bf16, isOutput=False)
    outT = nc.declare_dram_parameter("outT", [D, S], bf16, isOutput=True)

    with tile.TileContext(nc) as tc:
        import contextlib

        with contextlib.ExitStack() as ctx:
            sb = ctx.enter_context(tc.tile_pool(name="sb", bufs=1))
            ring = ctx.enter_context(tc.tile_pool(name="ring", bufs=4))
            ptpool = ctx.enter_context(tc.tile_pool(name="ptp", bufs=18))

            # ---- persistent SBUF tensors ----
            # load order tuned so proj_qk(0) / rope(0) / attention(0) can
            # start early: sync queue carries xT by 512-col phases; gpsimd
            # queue carries weights (wq/wk first) + rope tables.
            msk_sb = sb.tile([128, HPC, 128], bf16, tag="msk")
            nc.sync.dma_start(msk_sb, msk.rearrange("p (h c) -> p h c", h=HPC))
            xT_sb = sb.tile([128, NKT, S], bf16, tag="xT")
            xT_r = xT.rearrange("(o p) t -> p o t", p=128)
            for ph in range(NPH):
                csl = slice(ph * 512, (ph + 1) * 512)
                nc.sync.dma_start(xT_sb[:, :, csl], xT_r[:, :, csl])

            w_sb = {}
            for name, ap in (("q", wq), ("k", wk), ("v", wv)):
                t = sb.tile([128, NKT, DPC], bf16, tag=f"w{name}")
                nc.gpsimd.dma_start(t, ap.rearrange("(o p) m -> p o m", p=128))
                w_sb[name] = t
            bq_sb = sb.tile([128, 2], f32, tag="bq")
            nc.gpsimd.dma_start(bq_sb, bq.rearrange("m p -> p m"))
            bk_sb = sb.tile([128, 2], f32, tag="bk")
            nc.gpsimd.dma_start(bk_sb, bk.rearrange("m p -> p m"))
            bv_sb = sb.tile([128, DPC], f32, tag="bv")
            nc.gpsimd.dma_start(bv_sb, bv[:].to_broadcast((128, DPC)))
            cs_sb = sb.tile([128, S], bf16, tag="cs")
            nc.gpsimd.dma_start(cs_sb, cs[:])
            sn_sb = sb.tile([128, S], bf16, tag="sn")
            nc.gpsimd.dma_start(sn_sb, sn[:])
            wo_sb = sb.tile([128, 2, D], bf16, tag="wo")
            nc.sync.dma_start(wo_sb, wo.rearrange("(o p) m -> p o m", p=128))

            # raw (P1) and swapped (P2) projections, [128, m-block 2, S]
            P1 = {p: sb.tile([128, 2, S], bf16, tag=f"p1{p}", name=f"p1{p}")
                  for p in ("q", "k")}
            P2 = {p: sb.tile([128, 2, S], bf16, tag=f"p2{p}", name=f"p2{p}")
                  for p in ("q", "k")}

            # score MMs contract K=64 per head with tile_position row groups
            # (parity-0 heads rows 0-63, parity-1 rows 64-127): two heads run
            # concurrently in the array, no zero-padded q copies needed.
            # head->ss slot permutation puts concurrent pairs in different
            # PSUM banks: slot_of = [0, 2, 1, 3] (self-inverse).

            # v token-major with ones column (the softmax denominator rides
            # the PV matmul as output column HD): [128, HPC, 65]
            v_sb = [sb.tile([128, HPC, HD + 1], bf16, tag=f"v{t}", name=f"v{t}")
                    for t in range(NTOK)]
            for t in range(NTOK):
                nc.vector.memset(v_sb[t][:, :, HD:HD + 1], 1.0)

            # attn dims-major, [128 (2 heads x 64), S] bf16
            attnT = [sb.tile([128, S], bf16, tag=f"at{m}", name=f"at{m}")
                     for m in range(2)]

            psA = ctx.enter_context(tc.tile_pool(name="psA", bufs=2, space="PSUM"))
            psS = ctx.enter_context(tc.tile_pool(name="psS", bufs=2, space="PSUM"))
            psV = ctx.enter_context(tc.tile_pool(name="psV", bufs=1, space="PSUM"))

            def qk_chunk(ph, p, m):
                c0 = ph * 512
                csl = slice(c0, c0 + 512)
                bias = bq_sb if p == "q" else bk_sb
                ps = psA.tile([128, 512], f32, tag="ps")
                for kt in range(NKT):
                    nc.tensor.matmul(
                        ps,
                        lhsT=w_sb[p][:, kt, m * 128:(m + 1) * 128],
                        rhs=xT_sb[:, kt, csl],
                        start=(kt == 0), stop=(kt == NKT - 1),
                    )
                nc.vector.tensor_tensor(
                    P1[p][:, m, csl], ps,
                    bias[:, m:m + 1].to_broadcast((128, 512)),
                    ALU.add)

            def proj_qk(ph):
                for p in ("q", "k"):
                    for m in range(2):
                        qk_chunk(ph, p, m)
            def swap_phase(ph):
                c0 = ph * 512
                csl = slice(c0, c0 + 512)
                # swap 32-row blocks: P2 rows [a,b] <- P1 rows [b,a]
                q = nc.scalar if ph == 0 else nc.gpsimd
                for p in ("q", "k"):
                    for blk in range(4):
                        srow = blk * 32 + (32 if blk % 2 == 0 else -32)
                        q.dma_start(
                            P2[p][blk * 32:(blk + 1) * 32, :, csl],
                            P1[p][srow:srow + 32, :, csl])

            def rope_p(ph, p):
                c0 = ph * 512
                csl = slice(c0, c0 + 512)
                # rope: P1 = P1*cos + P2*(+-sin), in place
                for m in range(2):
                    nc.vector.tensor_mul(P1[p][:, m, csl], P1[p][:, m, csl],
                                         cs_sb[:, csl])
                    nc.vector.tensor_mul(P2[p][:, m, csl], P2[p][:, m, csl],
                                         sn_sb[:, csl])
                    nc.vector.tensor_add(P1[p][:, m, csl], P1[p][:, m, csl],
                                         P2[p][:, m, csl])

            def rope_phase(ph):
                swap_phase(ph)
                rope_p(ph, "q")
                rope_p(ph, "k")

            def proj_v(t0, t1):
                for t in range(t0, t1):
                    ps = psA.tile([128, 512], f32, tag="ps")
                    for kt in range(NKT):
                        nc.tensor.matmul(
                            ps[:, :DPC],
                            lhsT=xT_sb[:, kt, t * 128:(t + 1) * 128],
                            rhs=w_sb["v"][:, kt, :],
                            start=(kt == 0), stop=(kt == NKT - 1),
                        )
                    nc.vector.tensor_tensor(
                        v_sb[t][:, :, :HD],
                        ps[:, :DPC].rearrange("p (h d) -> p h d", h=HPC),
                        bv_sb.rearrange("p (h d) -> p h d", h=HPC),
                        ALU.add,
                    )
            def attention(tqg, fillers=()):
                fillers = list(fillers)
                q0 = tqg * 256
                nslots = 2 * tqg + 2
                pts = []
                pv = psV.tile([128, 2, 512], f32, tag="pv")
                at = ring.tile([128, 2, HPC, HD], bf16, tag="atm")

                def pv_mm(h, c, g, tkt):
                    last = 2 * tqg + c  # c=0 skips the off=128 slot
                    nc.tensor.matmul(
                        pv[:, g, :HD + 1],
                        lhsT=pts[tkt][:, SLOT_OF[h], c * 128:(c + 1) * 128],
                        rhs=v_sb[tkt][:, h, :],
                        start=(tkt == 0), stop=(tkt == last),
                    )

                def pv_drain(hp, c):
                    # normalize pair: divide by ones-column denominators
                    rec = ring.tile([128, 2, 1], f32, tag="rec")
                    nc.vector.reciprocal(rec, pv[:, :, HD:HD + 1])
                    nc.vector.tensor_tensor(
                        at[:, c, 2 * hp:2 * hp + 2, :], pv[:, :, :HD],
                        rec.to_broadcast((128, 2, HD)),
                        ALU.mult)
                    if hp == 1:  # both head-pairs of chunk c done
                        for mp in range(2):
                            # transposes stay off the scalar queue: a
                            # waiting transpose there head-of-line blocks
                            # the next exp (the attention pacer)
                            nc.sync.dma_start(
                                attnT[mp][:, q0 + c * 128:q0 + (c + 1) * 128],
                                at[:, c, 2 * mp:2 * mp + 2, :],
                                transpose=True)

                for tkt in range(nslots):
                    off = 128 if tkt == 2 * tqg + 1 else 0
                    ss = psS.tile([128, HPC, 256], f32, tag="ss")
                    for h in range(HPC):
                        j, par = h // 2, h % 2
                        s = SLOT_OF[h]
                        nc.tensor.matmul(
                            ss[:, s, off:],
                            lhsT=P1["k"][64 * par:64 * par + 64, j,
                                         tkt * 128:(tkt + 1) * 128],
                            rhs=P1["q"][64 * par:64 * par + 64, j,
                                        q0 + off:q0 + 256],
                            start=True, stop=True,
                        )
                    pt = ptpool.tile([128, HPC, 256], bf16, tag="pt")
                    pts.append(pt)
                    nc.scalar.activation(pt[:, :, off:], ss[:, :, off:],
                                         AF.Exp, scale=0.125)
                    if tkt >= 2 * tqg:  # diagonal slot: triangle mask
                        nc.vector.tensor_tensor(
                            pt[:, :, off:off + 128], pt[:, :, off:off + 128],
                            msk_sb, ALU.mult)
                    # first PV group pair (c=0, heads 0/1) rides the slot
                    # loop, lagging 2 slots so PE never waits on a fresh exp
                    if tkt - 2 >= 0 and tkt - 2 <= 2 * tqg:
                        for g in range(2):
                            pv_mm(g, 0, g, tkt - 2)
                    if fillers:
                        fillers.pop(0)()
                for tkt in range(2 * tqg if tqg else 0, 2 * tqg + 1):
                    for g in range(2):
                        pv_mm(g, 0, g, tkt)
                # remaining PV group pairs ping-pong across the 2 PSUM banks
                pv_drain(0, 0)
                for c, hp in ((0, 1), (1, 0), (1, 1)):
                    for g in range(2):
                        h = 2 * hp + g
                        for tkt in range(2 * tqg + c + 1):
                            pv_mm(h, c, g, tkt)
                    pv_drain(hp, c)
                for f in fillers:
                    f()

            def op_unit(q0, w, mo2):
                stg = ring.tile([128, 2, w], bf16, tag=f"stg{w}")
                for sub in range(2):
                    mo = mo2 * 2 + sub
                    ps = psA.tile([128, 512], f32, tag="ps")
                    for kt in range(2):
                        nc.tensor.matmul(
                            ps[:, :w],
                            lhsT=wo_sb[:, kt, mo * 128:(mo + 1) * 128],
                            rhs=attnT[kt][:, q0:q0 + w],
                            start=(kt == 0), stop=(kt == 1),
                        )
                    nc.vector.tensor_copy(stg[:, sub], ps[:, :w])
                nc.scalar.dma_start(
                    outT.rearrange("(o p) t -> p o t", p=128)[
                        :, mo2 * 2:mo2 * 2 + 2, q0:q0 + w],
                    stg)

            def outproj(q0, w):
                for mo2 in range(4):
                    op_unit(q0, w, mo2)

            # ---- emission schedule: proj/outproj units ride the
            # attention slot loops (PE slack under ACT-paced exp)
            from functools import partial

            proj_qk(0)
            rope_phase(0)
            proj_v(0, 4)
            attention(0)
            attention(1, [partial(qk_chunk, 1, "q", 0),
                          partial(qk_chunk, 1, "q", 1),
                          partial(qk_chunk, 1, "k", 0),
                          partial(qk_chunk, 1, "k", 1)])
            swap_phase(1)
            proj_v(4, 6)
            rope_p(1, "q")
            rope_p(1, "k")
            attention(2, [partial(proj_v, 6, 7), partial(proj_v, 7, 8),
                          partial(qk_chunk, 2, "q", 0),
                          partial(qk_chunk, 2, "q", 1),
                          partial(qk_chunk, 2, "k", 0),
                          partial(qk_chunk, 2, "k", 1)])
            attention(3, [partial(swap_phase, 2),
                          partial(rope_p, 2, "q"),
                          partial(rope_p, 2, "k"),
                          partial(proj_v, 8, 9), partial(proj_v, 9, 10)])
            attention(4, [partial(proj_v, 10, 11), partial(proj_v, 11, 12),
                          partial(qk_chunk, 3, "q", 0),
                          partial(qk_chunk, 3, "q", 1),
                          partial(qk_chunk, 3, "k", 0),
                          partial(qk_chunk, 3, "k", 1),
                          partial(op_unit, 0, 512, 0),
                          partial(op_unit, 0, 512, 1),
                          partial(op_unit, 0, 512, 2),
                          partial(op_unit, 0, 512, 3)])
            attention(5, [partial(swap_phase, 3),
                          partial(rope_p, 3, "q"),
                          partial(rope_p, 3, "k"),
                          partial(proj_v, 12, 13), partial(proj_v, 13, 14),
                          partial(proj_v, 14, 15), partial(proj_v, 15, 16),
                          partial(op_unit, 512, 512, 0),
                          partial(op_unit, 512, 512, 1),
                          partial(op_unit, 512, 512, 2),
                          partial(op_unit, 512, 512, 3)])
            attention(6, [partial(op_unit, 1024, 512, 0),
                          partial(op_unit, 1024, 512, 1),
                          partial(op_unit, 1024, 512, 2),
                          partial(op_unit, 1024, 512, 3)])
            attention(7, [partial(op_unit, 1536, 256, 0),
                          partial(op_unit, 1536, 256, 1),
                          partial(op_unit, 1536, 256, 2),
                          partial(op_unit, 1536, 256, 3)])
            outproj(1792, 256)

    nc.compile()
    return nc


def _prep(x, pos_cos, pos_sin, wq_w, wq_b, wk_w, wk_b, wv_w, wv_b, wo_w):
    """Build the 8 per-core input maps (numpy, host-side)."""
    bf = ml_dtypes.bfloat16
    # q/k d_out permutation: head-contiguous [a(32 even dims); b(32 odd)]
    perm = np.empty(DPC, dtype=np.int64)
    for hl in range(HPC):
        for i in range(HD // 2):
            perm[hl * 64 + i] = hl * HD + 2 * i
            perm[hl * 64 + 32 + i] = hl * HD + 2 * i + 1

    cosT = pos_cos.T.astype(np.float32)  # [32, S]
    sinT = pos_sin.T.astype(np.float32)
    blk_c = np.concatenate([cosT, cosT], 0)          # [64, S]
    blk_s = np.concatenate([-sinT, sinT], 0)         # [64, S]
    csT = np.ascontiguousarray(np.tile(blk_c, (2, 1))).astype(bf)  # [128, S]
    snT = np.ascontiguousarray(np.tile(blk_s, (2, 1))).astype(bf)
    tri = (np.arange(128)[None, :] >= np.arange(128)[:, None]).astype(bf)
    mask = np.ascontiguousarray(np.tile(tri, (1, HPC)))  # [128, 4*128]

    in_maps = []
    for c in range(NCORES):
        b, hg = divmod(c, HPC)
        sl = slice(hg * DPC, (hg + 1) * DPC)
        gperm = hg * DPC + perm
        m = {
            "xT": np.ascontiguousarray(x[b].T).astype(bf),
            "wq": np.ascontiguousarray(wq_w[gperm, :].T).astype(bf),
            "wk": np.ascontiguousarray(wk_w[gperm, :].T).astype(bf),
            "wv": np.ascontiguousarray(wv_w[sl, :].T).astype(bf),
            "bq": wq_b[gperm].reshape(2, 128).astype(np.float32),
            "bk": wk_b[gperm].reshape(2, 128).astype(np.float32),
            "bv": wv_b[sl].reshape(1, DPC).astype(np.float32),
            "cs": csT, "sn": snT, "msk": mask,
            "wo": np.ascontiguousarray(wo_w[:, sl].T).astype(bf),
        }
        in_maps.append(m)
    return in_maps


def kernel(x, pos_cos, pos_sin, wq_w, wq_b, wk_w, wk_b, wv_w, wv_b, wo_w, wo_b,
           _trace=False):
    from concourse.bass_utils import run_bass_kernel_spmd

    if "nc" not in _BUILT:
        _BUILT["nc"] = _build()
    nc = _BUILT["nc"]

    in_maps = _prep(x, pos_cos, pos_sin, wq_w, wq_b, wk_w, wk_b, wv_w, wv_b, wo_w)
    res = run_bass_kernel_spmd(nc, in_maps, core_ids=list(range(NCORES)),
                               trace=_trace)
    _BUILT["last"] = res

    out = np.empty((B, S, D), dtype=np.float32)
    for b in range(B):
        acc = res.results[b * HPC]["outT"].astype(np.float32)
        for hg in range(1, HPC):
            acc = acc + res.results[b * HPC + hg]["outT"].astype(np.float32)
        out[b] = acc.T + wo_b[None, :]
    return out



# revision 38
# speedup vs baseline: 1.1266x; 1.1266x over previous
"""Trainium2 Bass kernel for nn_Attention (B=2, S=2048, D=1024, H=16, hd=64).

Sharding: 8 cores = 2 batches x 4 head-groups (4 heads / 256 dims each).
Host sums the 4 partial output projections per batch and adds wo_b.

v6 design:
  - score MMs contract an honest K=64 per head via tile_position ROW
    groups (parity-0 heads rows 0-63, parity-1 rows 64-127); no
    zero-padded q copies. Head->ss slot permutation [0,2,1,3] keeps the
    row-group pair's outputs in different PSUM banks.
    (COL-tiled matmuls -- M<128 via tile_position[1] -- produce garbage
    on this toolchain; row tiling is fine. Verified empirically.)
  - RoPE full-width: P2 = 32-row-block-swapped copy of P1 (via sbuf-sbuf
    DMA), then q' = P1*cos + P2*(+-sin) -- 3 wide DVE ops per slice.
  - tq groups of 256; scores for all 4 heads of a (tqg, tkt) slot land in
    one [128, 4, 256] PSUM tile -> ONE exp per slot on ACT. All exps
    resolve to a single ACT table set (see _build's table patch).
  - PV token-major: out[tq 128, 65] via lhsT=probs chunk, rhs=v (ones
    column = softmax denominator -> per-PARTITION denominators, so the
    DVE reciprocal has free-size 2; DVE recip runs at 1/8 rate so wide
    reciprocals are catastrophic).
  - attn (token-major) -> dims-major via DMA-transpose XBAR, spread
    across the two HWDGE queues (sync + scalar); output projection per
    tqg interleaved with later attention; bf16 partials.
  - causal: skip above-diagonal tiles, 128-col trim on odd diag slots,
    triangle mask multiply on DVE.
  - prologue: xT loaded by 512-col phases on sync while weights/rope
    tables ride gpsimd, so proj/rope/attention(0) start ASAP.
"""

import sys

sys.path.insert(0, "/opt/trn_rl_repo")

import numpy as np
import ml_dtypes

B, S, D = 2, 2048, 1024
H = 16
HD = 64
HPC = 4          # heads per core
DPC = HPC * HD   # 256 dims per core
NCORES = 8
NKT = D // 128   # 8 k-tiles over d_in
NPH = 4          # projection phases (512 cols each)
NTQG = 8         # tq groups of 256
NTOK = S // 128  # 16 token tiles
SLOT_OF = [0, 2, 1, 3]  # head -> ss/pt slot (concurrent pairs in diff banks)

_BUILT = {}


def _build():
    import concourse.bass as bass
    import concourse.mybir as mybir
    import concourse.tile as tile
    from concourse import bacc

    dt = mybir.dt
    AF = mybir.ActivationFunctionType
    ALU = mybir.AluOpType

    nc = bacc.Bacc()

    # Exp and Ln both live in the natural_log_exp_and_others ACT table set.
    # The table chooser maps each func to the first set containing it, which
    # would alternate exp_and_others <-> natural_log (a ~2.7us table reload
    # per switch). Drop those funcs from the earlier sets (cached dict is
    # shared) so every activation resolves to the combined set: one load.
    from concourse.hw_specs import get_activation_tables
    tabs = get_activation_tables(nc.m.arch)
    if "natural_log_exp_and_others" in tabs:
        tabs["exp_and_others"].discard(AF.Exp)
        tabs["natural_log"].discard(AF.Ln)

    f32, bf16 = dt.float32, dt.bfloat16

    xT = nc.declare_dram_parameter("xT", [D, S], bf16, isOutput=False)
    wq = nc.declare_dram_parameter("wq", [D, DPC], bf16, isOutput=False)
    wk = nc.declare_dram_parameter("wk", [D, DPC], bf16, isOutput=False)
    wv = nc.declare_dram_parameter("wv", [D, DPC], bf16, isOutput=False)
    bq = nc.declare_dram_parameter("bq", [2, 128], f32, isOutput=False)
    bk = nc.declare_dram_parameter("bk", [2, 128], f32, isOutput=False)
    bv = nc.declare_dram_parameter("bv", [1, DPC], f32, isOutput=False)
    cs = nc.declare_dram_parameter("cs", [128, S], bf16, isOutput=False)
    sn = nc.declare_dram_parameter("sn", [128, S], bf16, isOutput=False)
    wo = nc.declare_dram_parameter("wo", [DPC, D], bf16, isOutput=False)
    msk = nc.declare_dram_parameter("msk", [128, HPC * 128], bf16, isOutput=False)
    outT = nc.declare_dram_parameter("outT", [D, S], bf16, isOutput=True)

    with tile.TileContext(nc) as tc:
        import contextlib

        with contextlib.ExitStack() as ctx:
            sb = ctx.enter_context(tc.tile_pool(name="sb", bufs=1))
            ring = ctx.enter_context(tc.tile_pool(name="ring", bufs=4))
            ptpool = ctx.enter_context(tc.tile_pool(name="ptp", bufs=18))

            # ---- persistent SBUF tensors ----
            # load order tuned so proj_qk(0) / rope(0) / attention(0) can
            # start early: sync queue carries xT by 512-col phases; gpsimd
            # queue carries weights (wq/wk first) + rope tables.
            xT_sb = sb.tile([128, NKT, S], bf16, tag="xT")
            xT_r = xT.rearrange("(o p) t -> p o t", p=128)
            for ph in range(NPH):
                csl = slice(ph * 512, (ph + 1) * 512)
                nc.sync.dma_start(xT_sb[:, :, csl], xT_r[:, :, csl])

            w_sb = {}
            for name, ap in (("q", wq), ("k", wk), ("v", wv)):
                t = sb.tile([128, NKT, DPC], bf16, tag=f"w{name}")
                nc.gpsimd.dma_start(t, ap.rearrange("(o p) m -> p o m", p=128))
                w_sb[name] = t
            bq_sb = sb.tile([128, 2], f32, tag="bq")
            nc.gpsimd.dma_start(bq_sb, bq.rearrange("m p -> p m"))
            bk_sb = sb.tile([128, 2], f32, tag="bk")
            nc.gpsimd.dma_start(bk_sb, bk.rearrange("m p -> p m"))
            bv_sb = sb.tile([128, DPC], f32, tag="bv")
            nc.gpsimd.dma_start(bv_sb, bv[:].to_broadcast((128, DPC)))
            cs_sb = sb.tile([128, S], bf16, tag="cs")
            nc.gpsimd.dma_start(cs_sb, cs[:])
            sn_sb = sb.tile([128, S], bf16, tag="sn")
            nc.gpsimd.dma_start(sn_sb, sn[:])
            msk_sb = sb.tile([128, HPC, 128], bf16, tag="msk")
            nc.gpsimd.dma_start(msk_sb, msk.rearrange("p (h c) -> p h c", h=HPC))
            wo_sb = sb.tile([128, 2, D], bf16, tag="wo")
            nc.gpsimd.dma_start(wo_sb, wo.rearrange("(o p) m -> p o m", p=128))

            # raw (P1) and swapped (P2) projections, [128, m-block 2, S]
            P1 = {p: sb.tile([128, 2, S], bf16, tag=f"p1{p}", name=f"p1{p}")
                  for p in ("q", "k")}
            P2 = {p: sb.tile([128, 2, S], bf16, tag=f"p2{p}", name=f"p2{p}")
                  for p in ("q", "k")}

            # score MMs contract K=64 per head with tile_position row groups
            # (parity-0 heads rows 0-63, parity-1 rows 64-127): two heads run
            # concurrently in the array, no zero-padded q copies needed.
            # head->ss slot permutation puts concurrent pairs in different
            # PSUM banks: slot_of = [0, 2, 1, 3] (self-inverse).

            # v token-major with ones column (the softmax denominator rides
            # the PV matmul as output column HD): [128, HPC, 65]
            v_sb = [sb.tile([128, HPC, HD + 1], bf16, tag=f"v{t}", name=f"v{t}")
                    for t in range(NTOK)]
            for t in range(NTOK):
                nc.vector.memset(v_sb[t][:, :, HD:HD + 1], 1.0)

            # attn dims-major, [128 (2 heads x 64), S] bf16
            attnT = [sb.tile([128, S], bf16, tag=f"at{m}", name=f"at{m}")
                     for m in range(2)]

            psA = ctx.enter_context(tc.tile_pool(name="psA", bufs=2, space="PSUM"))
            psS = ctx.enter_context(tc.tile_pool(name="psS", bufs=2, space="PSUM"))
            psV = ctx.enter_context(tc.tile_pool(name="psV", bufs=1, space="PSUM"))

            def qk_chunk(ph, p, m):
                c0 = ph * 512
                csl = slice(c0, c0 + 512)
                bias = bq_sb if p == "q" else bk_sb
                ps = psA.tile([128, 512], f32, tag="ps")
                for kt in range(NKT):
                    nc.tensor.matmul(
                        ps,
                        lhsT=w_sb[p][:, kt, m * 128:(m + 1) * 128],
                        rhs=xT_sb[:, kt, csl],
                        start=(kt == 0), stop=(kt == NKT - 1),
                    )
                nc.vector.tensor_tensor(
                    P1[p][:, m, csl], ps,
                    bias[:, m:m + 1].to_broadcast((128, 512)),
                    ALU.add)

            def proj_qk(ph):
                for p in ("q", "k"):
                    for m in range(2):
                        qk_chunk(ph, p, m)
            def swap_phase(ph):
                c0 = ph * 512
                csl = slice(c0, c0 + 512)
                # swap 32-row blocks: P2 rows [a,b] <- P1 rows [b,a]
                for p in ("q", "k"):
                    for blk in range(4):
                        srow = blk * 32 + (32 if blk % 2 == 0 else -32)
                        nc.gpsimd.dma_start(
                            P2[p][blk * 32:(blk + 1) * 32, :, csl],
                            P1[p][srow:srow + 32, :, csl])

            def rope_p(ph, p):
                c0 = ph * 512
                csl = slice(c0, c0 + 512)
                # rope: P1 = P1*cos + P2*(+-sin), in place
                for m in range(2):
                    nc.vector.tensor_mul(P1[p][:, m, csl], P1[p][:, m, csl],
                                         cs_sb[:, csl])
                    nc.vector.tensor_mul(P2[p][:, m, csl], P2[p][:, m, csl],
                                         sn_sb[:, csl])
                    nc.vector.tensor_add(P1[p][:, m, csl], P1[p][:, m, csl],
                                         P2[p][:, m, csl])

            def rope_phase(ph):
                swap_phase(ph)
                rope_p(ph, "q")
                rope_p(ph, "k")

            def proj_v(t0, t1):
                for t in range(t0, t1):
                    ps = psA.tile([128, 512], f32, tag="ps")
                    for kt in range(NKT):
                        nc.tensor.matmul(
                            ps[:, :DPC],
                            lhsT=xT_sb[:, kt, t * 128:(t + 1) * 128],
                            rhs=w_sb["v"][:, kt, :],
                            start=(kt == 0), stop=(kt == NKT - 1),
                        )
                    nc.vector.tensor_tensor(
                        v_sb[t][:, :, :HD],
                        ps[:, :DPC].rearrange("p (h d) -> p h d", h=HPC),
                        bv_sb.rearrange("p (h d) -> p h d", h=HPC),
                        ALU.add,
                    )
            def attention(tqg, fillers=()):
                fillers = list(fillers)
                q0 = tqg * 256
                nslots = 2 * tqg + 2
                pts = []
                pv = psV.tile([128, 2, 512], f32, tag="pv")
                at = ring.tile([128, 2, HPC, HD], bf16, tag="atm")

                def pv_mm(h, c, g, tkt):
                    last = 2 * tqg + c  # c=0 skips the off=128 slot
                    nc.tensor.matmul(
                        pv[:, g, :HD + 1],
                        lhsT=pts[tkt][:, SLOT_OF[h], c * 128:(c + 1) * 128],
                        rhs=v_sb[tkt][:, h, :],
                        start=(tkt == 0), stop=(tkt == last),
                    )

                def pv_drain(hp, c):
                    # normalize pair: divide by ones-column denominators
                    rec = ring.tile([128, 2, 1], f32, tag="rec")
                    nc.vector.reciprocal(rec, pv[:, :, HD:HD + 1])
                    nc.vector.tensor_tensor(
                        at[:, c, 2 * hp:2 * hp + 2, :], pv[:, :, :HD],
                        rec.to_broadcast((128, 2, HD)),
                        ALU.mult)
                    if hp == 1:  # both head-pairs of chunk c done
                        for mp in range(2):
                            # spread transpose XBAR DMAs across both HWDGE
                            # queues (sync + scalar); the at-tile is ready
                            # ~300ns after emission so the wait is short
                            q = nc.sync if mp == 0 else nc.scalar
                            q.dma_start(
                                attnT[mp][:, q0 + c * 128:q0 + (c + 1) * 128],
                                at[:, c, 2 * mp:2 * mp + 2, :],
                                transpose=True)

                for tkt in range(nslots):
                    off = 128 if tkt == 2 * tqg + 1 else 0
                    ss = psS.tile([128, HPC, 256], f32, tag="ss")
                    for h in range(HPC):
                        j, par = h // 2, h % 2
                        s = SLOT_OF[h]
                        nc.tensor.matmul(
                            ss[:, s, off:],
                            lhsT=P1["k"][64 * par:64 * par + 64, j,
                                         tkt * 128:(tkt + 1) * 128],
                            rhs=P1["q"][64 * par:64 * par + 64, j,
                                        q0 + off:q0 + 256],
                            start=True, stop=True,
                        )
                    pt = ptpool.tile([128, HPC, 256], bf16, tag="pt")
                    pts.append(pt)
                    nc.scalar.activation(pt[:, :, off:], ss[:, :, off:],
                                         AF.Exp, scale=0.125)
                    if tkt >= 2 * tqg:  # diagonal slot: triangle mask
                        nc.vector.tensor_tensor(
                            pt[:, :, off:off + 128], pt[:, :, off:off + 128],
                            msk_sb, ALU.mult)
                    # first PV group pair (c=0, heads 0/1) rides the slot
                    # loop, lagging 2 slots so PE never waits on a fresh exp
                    if tkt - 2 >= 0 and tkt - 2 <= 2 * tqg:
                        for g in range(2):
                            pv_mm(g, 0, g, tkt - 2)
                    if fillers:
                        fillers.pop(0)()
                for tkt in range(2 * tqg if tqg else 0, 2 * tqg + 1):
                    for g in range(2):
                        pv_mm(g, 0, g, tkt)
                # remaining PV group pairs ping-pong across the 2 PSUM banks
                pv_drain(0, 0)
                for c, hp in ((0, 1), (1, 0), (1, 1)):
                    for g in range(2):
                        h = 2 * hp + g
                        for tkt in range(2 * tqg + c + 1):
                            pv_mm(h, c, g, tkt)
                    pv_drain(hp, c)
                for f in fillers:
                    f()

            def op_unit(q0, w, mo2):
                stg = ring.tile([128, 2, w], bf16, tag=f"stg{w}")
                for sub in range(2):
                    mo = mo2 * 2 + sub
                    ps = psA.tile([128, 512], f32, tag="ps")
                    for kt in range(2):
                        nc.tensor.matmul(
                            ps[:, :w],
                            lhsT=wo_sb[:, kt, mo * 128:(mo + 1) * 128],
                            rhs=attnT[kt][:, q0:q0 + w],
                            start=(kt == 0), stop=(kt == 1),
                        )
                    nc.vector.tensor_copy(stg[:, sub], ps[:, :w])
                nc.sync.dma_start(
                    outT.rearrange("(o p) t -> p o t", p=128)[
                        :, mo2 * 2:mo2 * 2 + 2, q0:q0 + w],
                    stg)

            def outproj(q0, w):
                for mo2 in range(4):
                    op_unit(q0, w, mo2)

            # ---- emission schedule: proj/outproj units ride the
            # attention slot loops (PE slack under ACT-paced exp)
            from functools import partial

            proj_qk(0)
            proj_v(0, 4)
            rope_phase(0)
            attention(0)
            attention(1, [partial(qk_chunk, 1, "q", 0),
                          partial(qk_chunk, 1, "q", 1),
                          partial(qk_chunk, 1, "k", 0),
                          partial(qk_chunk, 1, "k", 1)])
            swap_phase(1)
            proj_v(4, 6)
            rope_p(1, "q")
            rope_p(1, "k")
            attention(2, [partial(proj_v, 6, 7), partial(proj_v, 7, 8),
                          partial(qk_chunk, 2, "q", 0),
                          partial(qk_chunk, 2, "q", 1),
                          partial(qk_chunk, 2, "k", 0),
                          partial(qk_chunk, 2, "k", 1)])
            attention(3, [partial(swap_phase, 2),
                          partial(rope_p, 2, "q"),
                          partial(rope_p, 2, "k"),
                          partial(proj_v, 8, 9), partial(proj_v, 9, 10)])
            attention(4, [partial(proj_v, 10, 11), partial(proj_v, 11, 12),
                          partial(qk_chunk, 3, "q", 0),
                          partial(qk_chunk, 3, "q", 1),
                          partial(qk_chunk, 3, "k", 0),
                          partial(qk_chunk, 3, "k", 1),
                          partial(op_unit, 0, 512, 0),
                          partial(op_unit, 0, 512, 1),
                          partial(op_unit, 0, 512, 2),
                          partial(op_unit, 0, 512, 3)])
            attention(5, [partial(swap_phase, 3),
                          partial(rope_p, 3, "q"),
                          partial(rope_p, 3, "k"),
                          partial(proj_v, 12, 13), partial(proj_v, 13, 14),
                          partial(proj_v, 14, 15), partial(proj_v, 15, 16),
                          partial(op_unit, 512, 512, 0),
                          partial(op_unit, 512, 512, 1),
                          partial(op_unit, 512, 512, 2),
                          partial(op_unit, 512, 512, 3)])
            attention(6, [partial(op_unit, 1024, 512, 0),
                          partial(op_unit, 1024, 512, 1),
                          partial(op_unit, 1024, 512, 2),
                          partial(op_unit, 1024, 512, 3)])
            attention(7, [partial(op_unit, 1536, 256, 0),
                          partial(op_unit, 1536, 256, 1),
                          partial(op_unit, 1536, 256, 2),
                          partial(op_unit, 1536, 256, 3)])
            outproj(1792, 256)

    nc.compile()
    return nc


def _prep(x, pos_cos, pos_sin, wq_w, wq_b, wk_w, wk_b, wv_w, wv_b, wo_w):
    """Build the 8 per-core input maps (numpy, host-side)."""
    bf = ml_dtypes.bfloat16
    # q/k d_out permutation: head-contiguous [a(32 even dims); b(32 odd)]
    perm = np.empty(DPC, dtype=np.int64)
    for hl in range(HPC):
        for i in range(HD // 2):
            perm[hl * 64 + i] = hl * HD + 2 * i
            perm[hl * 64 + 32 + i] = hl * HD + 2 * i + 1

    cosT = pos_cos.T.astype(np.float32)  # [32, S]
    sinT = pos_sin.T.astype(np.float32)
    blk_c = np.concatenate([cosT, cosT], 0)          # [64, S]
    blk_s = np.concatenate([-sinT, sinT], 0)         # [64, S]
    csT = np.ascontiguousarray(np.tile(blk_c, (2, 1))).astype(bf)  # [128, S]
    snT = np.ascontiguousarray(np.tile(blk_s, (2, 1))).astype(bf)
    tri = (np.arange(128)[None, :] >= np.arange(128)[:, None]).astype(bf)
    mask = np.ascontiguousarray(np.tile(tri, (1, HPC)))  # [128, 4*128]

    in_maps = []
    for c in range(NCORES):
        b, hg = divmod(c, HPC)
        sl = slice(hg * DPC, (hg + 1) * DPC)
        gperm = hg * DPC + perm
        m = {
            "xT": np.ascontiguousarray(x[b].T).astype(bf),
            "wq": np.ascontiguousarray(wq_w[gperm, :].T).astype(bf),
            "wk": np.ascontiguousarray(wk_w[gperm, :].T).astype(bf),
            "wv": np.ascontiguousarray(wv_w[sl, :].T).astype(bf),
            "bq": wq_b[gperm].reshape(2, 128).astype(np.float32),
            "bk": wk_b[gperm].reshape(2, 128).astype(np.float32),
            "bv": wv_b[sl].reshape(1, DPC).astype(np.float32),
            "cs": csT, "sn": snT, "msk": mask,
            "wo": np.ascontiguousarray(wo_w[:, sl].T).astype(bf),
        }
        in_maps.append(m)
    return in_maps


def kernel(x, pos_cos, pos_sin, wq_w, wq_b, wk_w, wk_b, wv_w, wv_b, wo_w, wo_b,
           _trace=False):
    from concourse.bass_utils import run_bass_kernel_spmd

    if "nc" not in _BUILT:
        _BUILT["nc"] = _build()
    nc = _BUILT["nc"]

    in_maps = _prep(x, pos_cos, pos_sin, wq_w, wq_b, wk_w, wk_b, wv_w, wv_b, wo_w)
    res = run_bass_kernel_spmd(nc, in_maps, core_ids=list(range(NCORES)),
                               trace=_trace)
    _BUILT["last"] = res

    out = np.empty((B, S, D), dtype=np.float32)
    for b in range(B):
        acc = res.results[b * HPC]["outT"].astype(np.float32)
        for hg in range(1, HPC):
            acc = acc + res.results[b * HPC + hg]["outT"].astype(np.float32)
        out[b] = acc.T + wo_b[None, :]
    return out

